# revision 25
# baseline (speedup 1.0000x reference)
"""Bahdanau additive attention, data-parallel over batch on 8 TRN2 NeuronCores.

Math (per batch row b):
    dec_proj = W @ prev[b] + b_W                       # [A]   (host: tiny)
    enc_proj[s] = U @ enc[b,s] + b_U                   # [S, A]
    energy[s] = v . tanh(dec_proj + enc_proj[s])       # [S]
    w = exp(energy);  c[b] = (w @ enc[b]) / sum(w)     # [CTX]

All-bf16 compute: fp8 (even DoubleRow, even only on the weighted-sum side)
fails the 2e-2 accuracy budget -- the softmax is highly concentrated
(eff-n ~ 7, energies span +-22), so per-element 3% quantization noise in
either the projection or the weighted sum lands at 3-10e-2 output error.

Per core (8 batches x 4096 s-rows), per 512-row super-tile:
  - enc natural bf16 -> nat [s=128, 4, 1024] (one 8KB/partition DMA,
    sync ring): rhs of the weighted-sum matmuls.
  - enc TRANSPOSED comes entirely from the HOST (enct, scalar ring);
    on-chip XBAR transposes are avoided: concurrent DmaTransposeAnt on
    the two HWDGE rings corrupts data, and serialized XBAR is
    packet-heavy and slower than just streaming the second layout from
    HBM (128 MiB total vs 96 MiB + 32 MiB through the XBAR).
  - projection per s-tile: ScalarE pre-writes the bias row into the PSUM
    tile (in-order with the tanh that frees the bank -- no WAR hazard),
    then 8 start=False matmuls tr[c,s].T @ ut[c,A] accumulate onto it.
  - tanh (ScalarE, psum->bf16) -> DVE tensor_mul by v_rep + DVE
    tensor_reduce -> energy column; one Exp per super-tile writes 4
    wbuf columns.
  - weighted sum runs D=6 s-tiles behind the projection so the
    tanh->mul->reduce->exp chain never stalls PE:
    c0/c1[1,512] += wbuf[:,j].T @ nat.  den = ones.T @ wbuf (one matmul
    per batch) + DVE reduce; epilogue c = c_psum * (1/den), DMA out f32.

PSUM: proj 3 banks rotating, c0/c1 double-buffered (4), den 1 = 8 total.
Engine budget per core: PE ~520K+262K cycles (proj+ws), ScalarE tanh+exp,
DVE mul/reduce/epilogue, both DMA rings ~64 MiB each.
"""

import sys

sys.path.insert(0, "/opt/trn_rl_repo")

import numpy as np
import ml_dtypes

import concourse.bass as bass
from concourse import bacc
import concourse.mybir as mybir
import concourse.tile as tile
from concourse.bass_utils import run_bass_kernel_spmd

B, S, A, DD, CTX = 64, 4096, 256, 1024, 1024
NCORES = 8
BL = B // NCORES   # 8 batches per core
P = 128
ST = 512           # s-rows per super-tile
NSUB = ST // P     # 4 s-subtiles per super-tile
NSUP = S // ST     # 8 super-tiles per batch
MT = S // P        # 32 s-tiles per batch
NT = BL * MT       # 256 s-tiles per core
NG = BL * NSUP     # 64 super-tiles per core
KC = CTX // P      # 8 contraction chunks
CH = CTX // 2      # host-transposed lower c-range (rest via XBAR)
KH = CH // P       # host-transposed k-chunks
BF16 = mybir.dt.bfloat16
F32 = mybir.dt.float32

D = 6              # ws pipeline delay in s-tiles
L = 2              # bias pre-emission lead (must be < psproj bufs - 1)
PF_LOAD = 4        # super-tiles of load prefetch
PF_TR = 2          # super-tiles of transpose lead
USE_PREWRITE = True  # bias via ScalarE PSUM pre-write (else bias matmul)
USE_TT = False       # energy via DVE tensor_tensor_reduce (else mul + accum)

_CACHE = {}


def _fast_bf16(x: np.ndarray) -> np.ndarray:
    """float32 -> bfloat16 (RNE). jax CPU cast is multithreaded; fall back
    to a vectorized integer path if jax is unavailable."""
    try:
        import jax, jax.numpy as jnp
        with jax.default_device(jax.devices("cpu")[0]):
            return np.asarray(jnp.asarray(x).astype(jnp.bfloat16))
    except Exception:
        u = np.ascontiguousarray(x, dtype=np.float32).view(np.uint32)
        r = ((u + 0x7FFF + ((u >> 16) & 1)) >> 16).astype(np.uint16)
        return r.view(ml_dtypes.bfloat16)


def _build():
    nc = bacc.Bacc()
    enc = nc.declare_dram_parameter("enc", [BL, S, CTX], BF16, isOutput=False)
    enct = nc.declare_dram_parameter("enct", [BL, NSUP, P, NSUB, KH, P],
                                     BF16, isOutput=False)
    ut = nc.declare_dram_parameter("ut", [P, KC, A], BF16, isOutput=False)
    biasr = nc.declare_dram_parameter("biasr", [P, BL, A], BF16, isOutput=False)
    dbrow = nc.declare_dram_parameter("dbrow", [1, BL * A], BF16, isOutput=False)
    vrep = nc.declare_dram_parameter("vrep", [P, A], BF16, isOutput=False)
    out = nc.declare_dram_parameter("out", [BL, CTX], F32, isOutput=True)

    with tile.TileContext(nc) as tc:
        with (
            tc.tile_pool(name="const", bufs=1) as const,
            tc.tile_pool(name="natp", bufs=8) as natp,
            tc.tile_pool(name="trhp", bufs=5) as trhp,
            tc.tile_pool(name="trxp", bufs=4) as trxp,
            tc.tile_pool(name="actp", bufs=3) as actp,
            tc.tile_pool(name="wbp", bufs=2) as wbp,
            tc.tile_pool(name="psproj", bufs=3, space="PSUM") as psproj,
            tc.tile_pool(name="psacc", bufs=2, space="PSUM") as psacc,
            tc.tile_pool(name="psden", bufs=1, space="PSUM") as psden,
        ):
            # ---- constants (issued after the first data tile below) ----
            ut_sb = const.tile([P, KC, A], BF16)
            bias_sb = const.tile([P, BL, A], BF16)
            v_sb = const.tile([P, A], BF16)
            ones_col = const.tile([P, 1], BF16)
            nc.vector.memset(ones_col[:], 1.0)
            db_sb = const.tile([1, BL * A], BF16)
            ones_row = const.tile([1, P], BF16)
            nc.vector.memset(ones_row[:], 1.0)
            scr = const.tile([P, 1], BF16)  # tt_reduce dump (broadcast out)

            nat = {}    # g -> [128, NSUB, CTX] bf16
            trh = {}    # g -> [128, NSUB, KC//2, P] bf16 (c in [0, 512))
            trx = {}    # g -> [128, NSUB, KC//2, P] bf16 (c in [512, 1024))
            projps = {} # i -> [128, A] f32 psum
            en = {}     # g -> [128, NSUB] f32
            wbuf = {}   # b -> [128, MT] bf16
            c0 = {}
            c1 = {}

            def issue_loads(g):
                if g >= NG:
                    return
                b, t = g // NSUP, g % NSUP
                nat[g] = natp.tile([P, NSUB, CTX], BF16, tag="nat", name=f"nat{g}")
                nc.sync.dma_start(
                    nat[g][:],
                    enc[b, t * ST:(t + 1) * ST, :].rearrange(
                        "(p o) c -> p o c", o=NSUB))
                trh[g] = trhp.tile([P, NSUB, KH, P], BF16, tag="trh", name=f"trh{g}")
                nc.scalar.dma_start(trh[g][:], enct[b, t])

            def issue_transposes(g):
                if g >= NG or KH == KC:
                    return
                trx[g] = trxp.tile([P, NSUB, KC - KH, P], BF16, tag="trx", name=f"trx{g}")
                for o in range(NSUB):
                    nc.scalar.dma_start_transpose(
                        trx[g][:, o, :, :], nat[g][:, o, CH:CTX])

            def prewrite(i):
                if i >= NT:
                    return
                b = i // MT
                projps[i] = psproj.tile([P, A], F32, tag="proj", name=f"proj{i}")
                if USE_PREWRITE:
                    nc.scalar.activation(projps[i][:], bias_sb[:, b, :],
                                         mybir.ActivationFunctionType.Copy)
                else:
                    nc.tensor.matmul(projps[i][:], ones_row[:],
                                     db_sb[:, b * A:(b + 1) * A],
                                     start=True, stop=False)

            # ---- warmup: first data tile ahead of the const uploads ----
            issue_loads(0)
            nc.sync.dma_start(ut_sb[:], ut[:])
            nc.scalar.dma_start(bias_sb[:], biasr[:])
            nc.scalar.dma_start(v_sb[:], vrep[:])
            nc.sync.dma_start(db_sb[:], dbrow[:])
            for g in range(1, PF_LOAD):
                issue_loads(g)
            for g in range(PF_TR):
                issue_transposes(g)
            for i in range(L):
                prewrite(i)

            for i in range(NT + D):
                if i < NT:
                    b, jj = i // MT, i % MT
                    g, o = i // NSUB, i % NSUB
                    if o == 0:
                        issue_loads(g + PF_LOAD)
                        issue_transposes(g + PF_TR)
                    prewrite(i + L)
                    # projection: 8 matmuls accumulate onto the bias
                    pp = projps[i]
                    for k in range(KC):
                        lhsT = (trh[g][:, o, k, :] if k < KH
                                else trx[g][:, o, k - KH, :])
                        nc.tensor.matmul(pp[:], lhsT, ut_sb[:, k, :],
                                         start=False, stop=(k == KC - 1),
                                         skip_group_check=USE_PREWRITE)
                    th = actp.tile([P, A], BF16, tag="th")
                    nc.scalar.activation(th[:], pp[:],
                                         mybir.ActivationFunctionType.Tanh)
                    if o == 0:
                        en[g] = actp.tile([P, NSUB], F32, tag="en",
                                          name=f"en{g}")
                    ew = actp.tile([P, A], BF16, tag="ew")
                    nc.vector.tensor_mul(out=ew[:], in0=th[:], in1=v_sb[:])
                    nc.vector.tensor_reduce(
                        en[g][:, o:o + 1], ew[:], axis=mybir.AxisListType.X,
                        op=mybir.AluOpType.add)
                    if jj == 0:
                        wbuf[b] = wbp.tile([P, MT], BF16, tag="wb", name=f"wb{b}")
                    if o == NSUB - 1:
                        t = g % NSUP
                        nc.scalar.activation(
                            wbuf[b][:, t * NSUB:(t + 1) * NSUB], en[g][:],
                            mybir.ActivationFunctionType.Exp)

                iw = i - D
                if iw >= 0:
                    bw, jw = iw // MT, iw % MT
                    gw, ow = iw // NSUB, iw % NSUB
                    if jw == 0:
                        c0[bw] = psacc.tile([1, 512], F32, tag="c0", name=f"c0_{bw}")
                        c1[bw] = psacc.tile([1, 512], F32, tag="c1", name=f"c1_{bw}")
                    first, last = (jw == 0), (jw == MT - 1)
                    wcol = wbuf[bw][:, jw:jw + 1]
                    nc.tensor.matmul(c0[bw][:], wcol, nat[gw][:, ow, 0:512],
                                     start=first, stop=last)
                    nc.tensor.matmul(c1[bw][:], wcol, nat[gw][:, ow, 512:1024],
                                     start=first, stop=last)
                    if last:
                        den = psden.tile([1, MT], F32, tag="den")
                        nc.tensor.matmul(den[:], ones_col[:], wbuf[bw][:],
                                         start=True, stop=True)
                        dsum = actp.tile([1, 1], F32, tag="dsum")
                        nc.vector.tensor_reduce(
                            dsum[:], den[:], axis=mybir.AxisListType.X,
                            op=mybir.AluOpType.add)
                        rec = actp.tile([1, 1], F32, tag="rec")
                        nc.vector.reciprocal(rec[:], dsum[:])
                        cout = actp.tile([1, CTX], F32, tag="cout")
                        nc.vector.tensor_scalar_mul(cout[:, 0:512], c0[bw][:],
                                                    rec[:])
                        nc.vector.tensor_scalar_mul(cout[:, 512:1024],
                                                    c1[bw][:], rec[:])
                        nc.sync.dma_start(out[bw][None, :], cout[:])

    if not nc.is_finalized():
        nc.finalize()
    return nc


def kernel(previous_decoder_hidden_state, encoder_final_hidden_layers,
           W, b_W, U, b_U, v):
    prev = np.asarray(previous_decoder_hidden_state, dtype=np.float32)
    enc = np.asarray(encoder_final_hidden_layers, dtype=np.float32)
    W = np.asarray(W, dtype=np.float32)
    b_W = np.asarray(b_W, dtype=np.float32)
    U = np.asarray(U, dtype=np.float32)
    b_U = np.asarray(b_U, dtype=np.float32)
    v = np.asarray(v, dtype=np.float32)

    if "nc" not in _CACHE:
        _CACHE["nc"] = _build()
    nc = _CACHE["nc"]

    # ---- host-side prep ----
    enc_bf = _fast_bf16(enc)                                  # [B, S, CTX]
    # host-transposed side: enct[b, t, q, o, k, j] = enc[b, t*512 + 4j + o,
    #                                                     k*128 + q]
    # s = t*512 + 4j + o  (p-major: nat partitions hold 4 contiguous s-rows)
    e = enc_bf[:, :, :CH].reshape(B, NSUP, P, NSUB, KH, P)
    enct = np.ascontiguousarray(e.transpose(0, 1, 5, 3, 4, 2))
    UT = np.ascontiguousarray(U.T)                            # [CTX, A]
    ut_host = np.ascontiguousarray(
        UT.reshape(KC, P, A).transpose(1, 0, 2)).astype(ml_dtypes.bfloat16)
    db = prev @ W.T + b_W + b_U                               # [B, A] f32
    db_bf = db.astype(ml_dtypes.bfloat16)
    v_host = np.ascontiguousarray(
        np.broadcast_to(v[None, :], (P, A))).astype(ml_dtypes.bfloat16)

    in_maps = []
    for i in range(NCORES):
        sl = slice(i * BL, (i + 1) * BL)
        biasr = np.ascontiguousarray(
            np.broadcast_to(db[sl][None, :, :],
                            (P, BL, A))).astype(ml_dtypes.bfloat16)
        in_maps.append({
            "enc": enc_bf[sl],
            "enct": enct[sl],
            "ut": ut_host,
            "biasr": biasr,
            "dbrow": db_bf[sl].reshape(1, BL * A),
            "vrep": v_host,
        })

    res = run_bass_kernel_spmd(nc, in_maps, list(range(NCORES)),
                               **_CACHE.get("run_kwargs", {}))
    _CACHE["last_result"] = res
    outs = [np.asarray(r["out"]) for r in res.results]
    return np.concatenate(outs, axis=0).astype(np.float32)
